# revision 6
# baseline (speedup 1.0000x reference)
"""Axial (frame-local) multi-head attention for Trainium2, 8-core SPMD.

Problem: x:[2,8192,512] -> qkv proj -> per-(batch,head,frame) attention over
n=1024 tokens -> out proj. B=2, f=8 frames, h=8 heads, d=64.

Sharding: the 16 (batch, frame) pairs are embarrassingly parallel; each of
the 8 cores handles 2 pairs end-to-end (weights replicated). Host
pre-transposes x so every on-chip matmul operand is naturally laid out.

Per-core pipeline (pair p, head-pair hp = heads 2hp,2hp+1):
  A:  qkT = [Wq*scale | Wk]^T @ xT   [1024ch, 1024tok] bf16 (ch-major)
      vv  = xT^T @ Wv                [1024tok, 512]    bf16 (tok-major)
  B:  per (hp, jt): simAB psum [128j, 2048 = 2 heads x 1024 i] via 4
      row-group-packed K=64 matmuls (2 heads concurrent);
      exp: ACT does 3 of 4 512-col chunks in ONE N=1536 activation, DVE
      does the 4th with a one-op Schraudolph (int16(a*x+b) bit-written
      into the bf16 et tile) -- splits the exp load across two engines;
      av: two col-tiled M=64 matmuls (both heads concurrent);
      den: four col-tiled M=1 ones-matmuls (all concurrent) accumulate
      softmax denominators into one PSUM bank at partitions 0/32/64/96.
      Per hp: consolidated reciprocal_approx_fast on [4,512], DRAM-bounce
      broadcast, fused PSUM-read normalize-multiply into bf16 otn.
  C:  y = otn^T @ Wout (+ bias via K=1 ones matmul), all-bf16 operands.

PSUM budget (8 banks): simAB 4 + po0 1 + po1 1 + pden 1 + pa (A/C) 1.
"""
import json
import numpy as np
from contextlib import ExitStack

import concourse.bass as bass
import concourse.tile as tile
import concourse.mybir as mybir
from concourse.bass_utils import run_bass_kernel_spmd

F32 = mybir.dt.float32
F32R = mybir.dt.float32r
BF16 = mybir.dt.bfloat16
I16 = mybir.dt.int16
AF = mybir.ActivationFunctionType
ALU = mybir.AluOpType

B, NTOT, DIM = 2, 8192, 512
H, D, F = 8, 64, 8
N = NTOT // F            # 1024 tokens per frame
SCALE = D ** -0.5
NP = 2                   # (batch, frame) pairs per core
TOK = NP * N             # 2048 tokens per core

# Schraudolph exp in bf16 bit space: bf16_bits(exp(x)) ~ round(A16*x + B16)
A16 = 2.0 ** 7 / np.log(2.0)      # 184.6650
B16 = 16256.0 - 7.5               # 0x3F80 minus RMS-optimal shift


def _legalize_waits(bir: bytes) -> bytes:
    """TRN2 instructions carry a single HW wait slot and this walrus build
    refuses to split multi-wait instructions; hoist extra waits onto NoOps
    inserted just before, on the same engine stream."""
    j = json.loads(bir)
    ctr = 0
    for fn in j["functions"]:
        for blk in fn["blocks"]:
            out = []
            for inst in blk["instructions"]:
                si = inst.get("sync_info")
                if si:
                    waits = si.get("on_wait") or []
                    if len(waits) > 1:
                        for w in waits[:-1]:
                            ctr += 1
                            nop = {
                                "engine": inst["engine"],
                                "ins": [], "outs": [],
                                "name": f"I-waitfix-{ctr}",
                                "opcode": "NoOp",
                                "sync_info": {"on_update": [], "on_wait": [w]},
                            }
                            if "debug" in inst:
                                nop["debug"] = inst["debug"]
                            out.append(nop)
                        si["on_wait"] = waits[-1:]
                out.append(inst)
            blk["instructions"] = out
    return json.dumps(j).encode()


def build(with_bias=True):
    nc = bass.Bass(trn_type="TRN2")
    xt = nc.dram_tensor("xt", [DIM, TOK], F32R, kind="ExternalInput")
    wqk = nc.dram_tensor("wqk", [DIM, 1024], F32R, kind="ExternalInput")
    wv = nc.dram_tensor("wv", [DIM, 512], F32R, kind="ExternalInput")
    wout = nc.dram_tensor("wout", [DIM, 512], BF16, kind="ExternalInput")
    bout = nc.dram_tensor("bout", [1, 512], BF16, kind="ExternalInput")
    y = nc.dram_tensor("y", [TOK, DIM], F32, kind="ExternalOutput")
    rscr = [nc.dram_tensor(f"rscr{t}", [4, 512], F32) for t in range(8)]

    with tile.TileContext(nc) as tc, ExitStack() as ctx:
        const = ctx.enter_context(tc.tile_pool(name="const", bufs=1))
        qk_pool = ctx.enter_context(tc.tile_pool(name="qk", bufs=2))
        vv_pool = ctx.enter_context(tc.tile_pool(name="vv", bufs=2))
        et_pool = ctx.enter_context(tc.tile_pool(name="et", bufs=3))
        otn_pool = ctx.enter_context(tc.tile_pool(name="otn", bufs=2))
        rd_pool = ctx.enter_context(tc.tile_pool(name="rd", bufs=2))
        y_pool = ctx.enter_context(tc.tile_pool(name="yo", bufs=2))
        den_pool2 = ctx.enter_context(tc.tile_pool(name="dsb", bufs=4))
        ps = ctx.enter_context(tc.tile_pool(name="ps", bufs=1, space="PSUM"))

        # ---- constants / weights (DMA-ordered by first use) ----
        wqk_sb = [const.tile([128, 1024], F32R, tag=f"wqk{k}", name=f"wqk{k}")
                  for k in range(4)]
        wv_sb = [const.tile([128, 512], F32R, tag=f"wv{k}", name=f"wv{k}")
                 for k in range(4)]
        wout_sb = [const.tile([128, 512], BF16, tag=f"wout{k}", name=f"wout{k}")
                   for k in range(4)]
        xt_sb = [const.tile([128, TOK], F32R, tag=f"xt{k}", name=f"xt{k}")
                 for k in range(4)]
        bout_sb = const.tile([1, 512], BF16, tag="bout", name="bout")

        # first A chains need wqk cols 0:512 (q) + xt cols 0:512
        for k in range(4):
            nc.sync.dma_start(wqk_sb[k][:, 0:512],
                              wqk.ap()[k * 128:(k + 1) * 128, 0:512])
            nc.sync.dma_start(xt_sb[k][:, 0:512],
                              xt.ap()[k * 128:(k + 1) * 128, 0:512])
        for k in range(4):
            nc.sync.dma_start(xt_sb[k][:, 512:N],
                              xt.ap()[k * 128:(k + 1) * 128, 512:N])
        for k in range(4):
            nc.sync.dma_start(wqk_sb[k][:, 512:1024],
                              wqk.ap()[k * 128:(k + 1) * 128, 512:1024])
        for k in range(4):
            nc.sync.dma_start(wv_sb[k][:], wv.ap()[k * 128:(k + 1) * 128, :])
        for k in range(4):
            nc.sync.dma_start(wout_sb[k][:], wout.ap()[k * 128:(k + 1) * 128, :])
        for k in range(4):
            nc.sync.dma_start(xt_sb[k][:, N:N + 512],
                              xt.ap()[k * 128:(k + 1) * 128, N:N + 512])
        for k in range(4):
            nc.sync.dma_start(xt_sb[k][:, N + 512:TOK],
                              xt.ap()[k * 128:(k + 1) * 128, N + 512:TOK])
        nc.sync.dma_start(bout_sb[:], bout.ap())

        ones_f = const.tile([128, 128], F32, tag="ones_f", name="ones_f")
        nc.gpsimd.memset(ones_f[:], 1.0)
        ones_b = const.tile([128, 128], BF16, tag="ones_b", name="ones_b")
        nc.gpsimd.memset(ones_b[:], 1.0)
        # warm the ACT exp table set during the startup DMA window
        warm = const.tile([1, 8], F32, tag="warm", name="warm")
        nc.scalar.activation(warm[:], ones_f[0:1, 0:8], AF.Exp)

        S = {0: {}, 1: {}}

        def emit_A(pi):
            t0 = pi * N
            # ---- qkT [1024ch, 1024tok], ch-major, bf16 out ----
            S[pi]['qkT'] = qkT = [
                qk_pool.tile([128, N], BF16, tag=f"qkT{c}", name=f"qkT{c}")
                for c in range(8)]
            for cht in range(8):
                for nt in range(2):
                    pa = ps.tile([128, 512], F32, tag="pa", name="pa")
                    for kt in range(4):
                        nc.tensor.matmul(
                            pa[:],
                            wqk_sb[kt][:, cht * 128:(cht + 1) * 128],
                            xt_sb[kt][:, t0 + nt * 512:t0 + (nt + 1) * 512],
                            start=(kt == 0), stop=(kt == 3))
                    nc.vector.tensor_copy(qkT[cht][:, nt * 512:(nt + 1) * 512],
                                          pa[:])

            # ---- v tok-major bf16 [128 tok, 512 dims] per token block ----
            S[pi]['vv'] = vv = [
                vv_pool.tile([128, 512], BF16, tag=f"vv{t}", name=f"vv{t}")
                for t in range(8)]
            for tt in range(8):
                pv = ps.tile([128, 512], F32, tag="pa", name="pa")
                for kt in range(4):
                    nc.tensor.matmul(
                        pv[:],
                        xt_sb[kt][:, t0 + tt * 128:t0 + (tt + 1) * 128],
                        wv_sb[kt][:],
                        start=(kt == 0), stop=(kt == 3))
                nc.vector.tensor_copy(vv[tt][:], pv[:])

        def emit_B(pi):
            qkT = S[pi]['qkT']; vv = S[pi]['vv']
            S[pi]['otn'] = otn = [
                otn_pool.tile([128, N], BF16, tag=f"otn{t}", name=f"otn{t}")
                for t in range(4)]
            for hp in range(4):
                qt, kt_ = hp, 4 + hp
                hA, hB = 2 * hp, 2 * hp + 1
                po = [ps.tile([128, 512], F32, tag=f"po{i}", name=f"po{i}")
                      for i in (0, 1)]
                pden = ps.tile([128, 512], F32, tag="pden", name="pden")
                # fill the bank so the one-shot reciprocal below reads no
                # uninitialized PSUM (only rows 0/32/64/96 carry real sums)
                nc.vector.memset(pden[:], 1.0)
                for jt in range(8):
                    sim = ps.tile([128, 2048], F32, tag="simAB", name="simAB")
                    for it in range(2):
                        nc.tensor.matmul(
                            sim[:, it * 512:(it + 1) * 512],
                            qkT[kt_][0:64, jt * 128:(jt + 1) * 128],
                            qkT[qt][0:64, it * 512:(it + 1) * 512],
                            start=True, stop=True, tile_position=(0, 0))
                        nc.tensor.matmul(
                            sim[:, 1024 + it * 512:1024 + (it + 1) * 512],
                            qkT[kt_][64:128, jt * 128:(jt + 1) * 128],
                            qkT[qt][64:128, it * 512:(it + 1) * 512],
                            start=True, stop=True, tile_position=(64, 0))
                    et = et_pool.tile([128, 2048], BF16, tag="et", name="et")
                    # exp: DVE takes one end chunk (Schraudolph bit-trick),
                    # ACT the remaining 1536 cols in a single activation
                    c = 0 if (jt + hp) % 2 == 0 else 3
                    nc.vector.tensor_scalar(
                        et[:, c * 512:(c + 1) * 512].bitcast(I16),
                        sim[:, c * 512:(c + 1) * 512],
                        A16, B16, ALU.mult, ALU.add)
                    if c == 0:
                        nc.scalar.activation(et[:, 512:2048],
                                             sim[:, 512:2048], AF.Exp)
                    else:
                        nc.scalar.activation(et[:, 0:1536],
                                             sim[:, 0:1536], AF.Exp)
                    # av: both heads concurrent via column tiling
                    for it in range(2):
                        nc.tensor.matmul(
                            po[it][0:64, :],
                            vv[jt][:, hA * 64:(hA + 1) * 64],
                            et[:, it * 512:(it + 1) * 512],
                            start=(jt == 0), stop=(jt == 7),
                            tile_position=(0, 0))
                        nc.tensor.matmul(
                            po[it][64:128, :],
                            vv[jt][:, hB * 64:(hB + 1) * 64],
                            et[:, 1024 + it * 512:1024 + (it + 1) * 512],
                            start=(jt == 0), stop=(jt == 7),
                            tile_position=(0, 64))
                    # den: 4 concurrent M=1 ones-matmuls, one per 512-chunk
                    for r in range(4):
                        nc.tensor.matmul(
                            pden[32 * r:32 * r + 1, :],
                            ones_b[:, 0:1],
                            et[:, r * 512:(r + 1) * 512],
                            start=(jt == 0), stop=(jt == 7),
                            tile_position=(0, 32 * r))
                # consolidated per-hp denominators -> reciprocal straight
                # from PSUM in one op; compact rows at the DMA
                rcp = rd_pool.tile([128, 512], F32, tag="rcp", name="rcp")
                nc.vector.reciprocal(rcp[:], pden[:])
                sc = rscr[pi * 4 + hp]
                for r in range(4):
                    nc.sync.dma_start(sc.ap()[r:r + 1, :],
                                      rcp[32 * r:32 * r + 1, :])
                for it in range(2):
                    den_sb = den_pool2.tile([128, 512], F32, tag="dsb",
                                            name="dsb")
                    nc.sync.dma_start(
                        den_sb[0:64, :],
                        sc.ap()[it:it + 1, :].broadcast_to([64, 512]))
                    nc.sync.dma_start(
                        den_sb[64:128, :],
                        sc.ap()[2 + it:3 + it, :].broadcast_to([64, 512]))
                    # fused PSUM eviction + normalization -> bf16 otn
                    nc.vector.tensor_tensor(
                        otn[hp][:, it * 512:(it + 1) * 512],
                        po[it][:], den_sb[:], ALU.mult)

        def emit_C(pi):
            otn = S[pi]['otn']
            tags = ["pa"] if pi == 0 else ["pa", "po0", "po1", "pden"]
            for tt in range(8):
                tg = tags[tt % len(tags)]
                py = ps.tile([128, 512], F32, tag=tg, name="py")
                if with_bias:
                    nc.tensor.matmul(py[:], ones_b[0:1, :], bout_sb[:],
                                     start=True, stop=False,
                                     tile_position=(0, 0))
                for kt in range(4):
                    nc.tensor.matmul(
                        py[:],
                        otn[kt][:, tt * 128:(tt + 1) * 128],
                        wout_sb[kt][:],
                        start=(not with_bias and kt == 0), stop=(kt == 3))
                ysb = y_pool.tile([128, 512], F32, tag="ysb", name="ysb")
                if pi == 1:
                    nc.scalar.copy(ysb[:], py[:])  # ACT is idle at the tail
                else:
                    nc.vector.tensor_copy(ysb[:], py[:])
                nc.sync.dma_start(
                    y.ap()[pi * N + tt * 128:pi * N + (tt + 1) * 128, :],
                    ysb[:])

        emit_A(0)
        emit_B(0)
        emit_A(1)
        emit_C(0)
        emit_B(1)
        emit_C(1)

    _orig = nc.to_json_bytes
    nc.to_json_bytes = lambda: _legalize_waits(_orig())
    return nc


_NC_CACHE = []
_last_in_maps = None


def kernel(**inputs) -> np.ndarray:
    import ml_dtypes
    x = np.ascontiguousarray(np.asarray(inputs["x"], dtype=np.float32))
    W_qkv = np.asarray(inputs["W_qkv"], dtype=np.float32)
    W_out = np.ascontiguousarray(np.asarray(inputs["W_out"], dtype=np.float32))
    b_out = np.ascontiguousarray(np.asarray(inputs["b_out"], dtype=np.float32))
    f = int(np.asarray(inputs["f"]))
    assert f == F and x.shape == (B, NTOT, DIM)

    Wqk = np.ascontiguousarray(
        np.concatenate([W_qkv[:, :512] * SCALE, W_qkv[:, 512:1024]], axis=1))
    Wv = np.ascontiguousarray(W_qkv[:, 1024:1536])
    Wo = np.ascontiguousarray(W_out.astype(ml_dtypes.bfloat16))
    bo = np.ascontiguousarray(b_out.reshape(1, 512).astype(ml_dtypes.bfloat16))

    with_bias = bool(np.any(b_out))
    key = with_bias
    if not _NC_CACHE or _NC_CACHE[0][0] != key:
        _NC_CACHE.clear()
        _NC_CACHE.append((key, build(with_bias)))
    nc = _NC_CACHE[0][1]

    in_maps = []
    for core in range(8):
        pairs = (2 * core, 2 * core + 1)
        xT = np.concatenate(
            [x[p // F, (p % F) * N:(p % F + 1) * N, :].T for p in pairs], axis=1)
        in_maps.append({
            "xt": np.ascontiguousarray(xT),
            "wqk": Wqk, "wv": Wv, "wout": Wo, "bout": bo,
        })

    global _last_in_maps
    _last_in_maps = in_maps
    try:
        res = run_bass_kernel_spmd(nc, in_maps, list(range(8)))
    except Exception:
        # transient NRT_EXEC_UNIT_UNRECOVERABLE occasionally hits the first
        # submission after a fresh compile; one retry has always cleared it
        import time
        time.sleep(10)
        res = run_bass_kernel_spmd(nc, in_maps, list(range(8)))

    out = np.zeros((B, NTOT, DIM), dtype=np.float32)
    for core in range(8):
        yc = res.results[core]["y"]
        for pi, p in enumerate((2 * core, 2 * core + 1)):
            out[p // F, (p % F) * N:(p % F + 1) * N, :] = yc[pi * N:(pi + 1) * N]
    return out
